# revision 7
# baseline (speedup 1.0000x reference)
"""Space-to-depth (8x8 chessboard) kernel for Trainium2.

Full input  : (32, 256, 256, 32) f32
Full output : (32, 8, 8, 32768) f32
out[b, i, j] = inputs[b, i*32:(i+1)*32, j*32:(j+1)*32, :].reshape(-1)

Sharding: batch dim (32) split across 8 NeuronCores (pure data parallel,
no communication) -> 4 examples per core.

Pure HBM->HBM data movement via DMA access patterns (no compute
engines). Within one (example b, 32-row band i), iterating (r, j, elem)
makes the source AP contiguous and the destination a 3D AP, so a single
DMA moves a half-band (16 rows) in 4 KiB chunks:

  src [[8192, 16], [1024, 8], [1, 1024]]   (contiguous 32 KiB per row r)
  dst [[1024, 16], [32768, 8], [1, 1024]]  (4 KiB chunks, 128 KiB stride)

Measured facts driving this schedule (trn2 NTFF traces):
- ~315-320 GB/s payload per core is a hard wall: a plain contiguous
  HBM->HBM copy measures the same 114-117 us regardless of descriptor
  size (2-32 KiB), i.e. 16 SDMA engines x ~20 GB/s sustained. The
  permutation itself is free; this kernel runs at that roofline.
- SDMA engine = (outer AP dim index) mod 16; keep HWDGE DMAs at
  <= 128 descriptors (outer <= 16) - larger hits a slow DGE fallback.
- Issue from both HWDGE queues (sync=SP + scalar=ACT).
- Engines 0-4 receive their first descriptors ~3.4 us before engines
  5-15 (first-instruction descriptor-generation ramp), so engines 0-4
  get two extra units. The first job per queue is split into four
  32-descriptor instructions to shorten that ramp.
- On this dst pattern engine 15 streams ~1.24x slower (quirk absent in
  contiguous copies): 12 half-band jobs carry 15 rows instead of 16,
  and the 12 skipped rows are covered by two early orphan DMAs whose
  outer counts (7 and 5) land them on engines 0-6 / 0-4.
- Per-engine loads: eng0-4: 528 descs, eng5-6: 520, eng7-14: 512,
  eng15: 416 -> all engines finish within ~1.5 us of each other.
"""

import numpy as np

_B_PER_CORE = 4
_N_CORES = 8
_IN_SHAPE = (_B_PER_CORE, 256, 256, 32)
_OUT_SHAPE = (_B_PER_CORE, 8, 8, 32768)
_EX = 256 * 256 * 32      # elements per example  (2097152)
_BAND = 32 * 256 * 32     # elements per (example, row-band)  (262144)

_CACHE = {}


def build_nc():
    import concourse.bass as bass
    import concourse.mybir as mybir

    nc = bass.Bass(target_bir_lowering=False)
    x = nc.dram_tensor("x", list(_IN_SHAPE), mybir.dt.float32, kind="ExternalInput")
    y = nc.dram_tensor("y", list(_OUT_SHAPE), mybir.dt.float32, kind="ExternalOutput")

    # Half-band jobs keyed by global band g = 8*b + i.  Jobs with g < 12,
    # h = 1 carry 15 rows (engine-15 skew); their skipped row 31 is
    # covered by the two orphan jobs.
    jobs_all = [
        (g, h * 16, 15 if (h == 1 and g < 12) else 16)
        for g in range(32)
        for h in range(2)
    ]
    q1_jobs = jobs_all[0::2]
    q2_jobs = jobs_all[1::2]
    # orphans: row 31 of bands 0-6 (outer 7 -> eng0-6) and 7-11 (outer 5
    # -> eng0-4), issued early so their uneven tail lands mid-stream.
    orph1 = ("orph", 0, 7)
    orph2 = ("orph", 7, 5)
    q1_jobs = q1_jobs[:1] + [orph1] + q1_jobs[1:]
    q2_jobs = q2_jobs[:1] + [orph2] + q2_jobs[1:]

    def issue(engine, my_jobs, sem):
        n = 0
        for k, job in enumerate(my_jobs):
            if job[0] == "orph":
                _, g0, cnt = job
                src = bass.AP(
                    x, g0 * _BAND + 31 * 8192, [[_BAND, cnt], [1024, 8], [1, 1024]]
                )
                dst = bass.AP(
                    y, g0 * _BAND + 31 * 1024, [[_BAND, cnt], [32768, 8], [1, 1024]]
                )
                engine.dma_start(out=dst, in_=src).then_inc(sem, 16)
                n += 16
                continue
            g, r0, nr = job
            off = g * _BAND
            if k == 0:
                # warmup: four 32-descriptor instructions (2 j's each) so
                # all 16 engines start streaming ~3 us sooner
                for j0 in range(0, 8, 2):
                    src = bass.AP(
                        x,
                        off + r0 * 8192 + j0 * 1024,
                        [[8192, nr], [1024, 2], [1, 1024]],
                    )
                    dst = bass.AP(
                        y,
                        off + r0 * 1024 + j0 * 32768,
                        [[1024, nr], [32768, 2], [1, 1024]],
                    )
                    engine.dma_start(out=dst, in_=src).then_inc(sem, 16)
                    n += 16
                continue
            src = bass.AP(x, off + r0 * 8192, [[8192, nr], [1024, 8], [1, 1024]])
            dst = bass.AP(y, off + r0 * 1024, [[1024, nr], [32768, 8], [1, 1024]])
            engine.dma_start(out=dst, in_=src).then_inc(sem, 16)
            n += 16
        if n:
            engine.wait_ge(sem, n)

    with (
        nc.semaphore("sp_sem") as sp_sem,
        nc.semaphore("act_sem") as act_sem,
        nc.Block(no_gpsimd_drain=True) as block,
    ):

        @block.sync
        def _(sync):
            issue(sync, q1_jobs, sp_sem)

        @block.scalar
        def _(scalar):
            issue(scalar, q2_jobs, act_sem)

    return nc


def _get_nc():
    if "nc" not in _CACHE:
        _CACHE["nc"] = build_nc()
    return _CACHE["nc"]


def kernel(inputs: np.ndarray) -> np.ndarray:
    from concourse.bass_utils import run_bass_kernel_spmd

    inputs = np.ascontiguousarray(np.asarray(inputs, dtype=np.float32))
    assert inputs.shape == (_B_PER_CORE * _N_CORES,) + _IN_SHAPE[1:]

    nc = _get_nc()
    in_maps = [
        {"x": np.ascontiguousarray(inputs[c * _B_PER_CORE : (c + 1) * _B_PER_CORE])}
        for c in range(_N_CORES)
    ]
    res = run_bass_kernel_spmd(nc, in_maps, core_ids=list(range(_N_CORES)))
    return np.concatenate([r["y"] for r in res.results], axis=0)


# revision 9
# speedup vs baseline: 1.0241x; 1.0241x over previous
"""Space-to-depth (8x8 chessboard) kernel for Trainium2.

Full input  : (32, 256, 256, 32) f32
Full output : (32, 8, 8, 32768) f32
out[b, i, j] = inputs[b, i*32:(i+1)*32, j*32:(j+1)*32, :].reshape(-1)

Sharding: batch dim (32) split across 8 NeuronCores (pure data parallel,
no communication) -> 4 examples per core.

Per core the op is pure HBM->HBM data movement, done entirely with DMA
access patterns (no compute engines). Key layout fact: within one
(example b, 32-row band i), iterating (r, j, elem) makes the source AP
contiguous and the destination a 3D AP, so a single DMA moves a
half-band (16 rows = 512 KiB) in 4 KiB contiguous chunks:

  src [[8192, nr], [1024, 8], [1, 1024]]   (contiguous 32 KiB per row r)
  dst [[1024, nr], [32768, 8], [1, 1024]]  (4 KiB chunks, 32 KiB stride)

Performance notes (measured on trn2 via NTFF traces):
- SDMA engine assignment is (outer AP dim index) mod 16, so outer count
  >= 16 engages all 16 SDMA engines (outer 8 uses only engines 0-7).
- Keep HWDGE DMAs at <= 128 descriptors (outer <= 16): outer 31/32 DMAs
  hit a slow descriptor-generation fallback that blocks the issuing
  sequencer 10-100 us per instruction and starves the engines (6x slower).
- 4 KiB descriptors outperform 32 KiB ones (~320 vs ~213 GB/s payload).
- Issuing from both HWDGE queues (sync=SP + scalar=ACT) beats one queue.
- SDMA engine 15 is intermittently ~1.25x slower (known trn2 quirk), so
  the job list is skewed: 16 of the 64 half-band DMAs carry 15 rows
  instead of 16 (their unit 15 would land on engine 15), and the 16
  skipped rows are covered by two batched "orphan" DMAs whose outer
  count 8 lands on engines 0-7. Engine 15 ends up with ~73% of the
  average load and no longer straggles.

Steady state ~320 GB/s payload (~640 GB/s HBM read+write traffic per
core) with all 8 cores running - measured equal to a plain contiguous
HBM->HBM copy, i.e. the permutation itself is free and the kernel runs
at the achievable DMA/HBM roofline. HW exec ~115-118 us per core.
"""

import numpy as np

_B_PER_CORE = 4
_N_CORES = 8
_IN_SHAPE = (_B_PER_CORE, 256, 256, 32)
_OUT_SHAPE = (_B_PER_CORE, 8, 8, 32768)
_EX = 256 * 256 * 32      # elements per example  (2097152)
_BAND = 32 * 256 * 32     # elements per (example, row-band)  (262144)

_CACHE = {}


def build_nc():
    import concourse.bass as bass
    import concourse.mybir as mybir

    nc = bass.Bass(target_bir_lowering=False)
    x = nc.dram_tensor("x", list(_IN_SHAPE), mybir.dt.float32, kind="ExternalInput")
    y = nc.dram_tensor("y", list(_OUT_SHAPE), mybir.dt.float32, kind="ExternalOutput")

    # Job list: half-band DMAs with the two orphan batches issued second
    # (not last): the orphans' outer count 8 lands only on engines 0-7,
    # so issuing them at the end would serialize a ~2.5 us tail onto
    # those engines while 8-15 sit idle. For b in {0,1} the second
    # half-band is shortened to 15 rows (engine-15 skew).
    jobs = [
        (b, i, h * 16, 15 if (h == 1 and b < 2) else 16)
        for b in range(_B_PER_CORE)
        for i in range(8)
        for h in range(2)
    ]
    jobs = jobs[:2] + [("orph", 0), ("orph", 1)] + jobs[2:]

    def issue(engine, my_jobs, sem):
        n = 0
        for job in my_jobs:
            if job[0] == "orph":
                # rows r=31 of all 8 bands of example b; one 32 KiB unit
                # per band -> outer count 8 -> SDMA engines 0-7
                _, b = job
                src = bass.AP(
                    x, b * _EX + 31 * 8192, [[262144, 8], [1024, 8], [1, 1024]]
                )
                dst = bass.AP(
                    y, b * _EX + 31 * 1024, [[262144, 8], [32768, 8], [1, 1024]]
                )
            else:
                b, i, r0, nr = job
                off = b * _EX + i * _BAND
                src = bass.AP(
                    x, off + r0 * 8192, [[8192, nr], [1024, 8], [1, 1024]]
                )
                dst = bass.AP(
                    y, off + r0 * 1024, [[1024, nr], [32768, 8], [1, 1024]]
                )
            engine.dma_start(out=dst, in_=src).then_inc(sem, 16)
            n += 16
        if n:
            engine.wait_ge(sem, n)

    with (
        nc.semaphore("sp_sem") as sp_sem,
        nc.semaphore("act_sem") as act_sem,
        nc.Block(no_gpsimd_drain=True) as block,
    ):

        @block.sync
        def _(sync):
            issue(sync, jobs[0::2], sp_sem)

        @block.scalar
        def _(scalar):
            issue(scalar, jobs[1::2], act_sem)

    return nc


def _get_nc():
    if "nc" not in _CACHE:
        _CACHE["nc"] = build_nc()
    return _CACHE["nc"]


def kernel(inputs: np.ndarray) -> np.ndarray:
    from concourse.bass_utils import run_bass_kernel_spmd

    inputs = np.ascontiguousarray(np.asarray(inputs, dtype=np.float32))
    assert inputs.shape == (_B_PER_CORE * _N_CORES,) + _IN_SHAPE[1:]

    nc = _get_nc()
    in_maps = [
        {"x": np.ascontiguousarray(inputs[c * _B_PER_CORE : (c + 1) * _B_PER_CORE])}
        for c in range(_N_CORES)
    ]
    res = run_bass_kernel_spmd(nc, in_maps, core_ids=list(range(_N_CORES)))
    return np.concatenate([r["y"] for r in res.results], axis=0)

